# revision 7
# baseline (speedup 1.0000x reference)
"""DilateAttention3D (3x3x3 window, dil=1) Trainium2 Bass kernel, 8-core SPMD.

Sharding: core = (b, dc) for b in {0,1}, dc in {0..3}: one batch element and a
D-chunk of 4 (halo 1 from zero-padded k/v) per core.

Per-core tile = (dz, y, xh): 6 heads x 16 queries = 96 voxels, key union
F = 3*3*18 = 162 positions (2 chunks of 81). Host pre-gathers a contiguous
per-tile k-window and v-window-transposed (col 96 = ones -> denominator).

The out-of-window additive mask (-BIG on the 162-union entries outside each
query's 27-window) has rank <= 17 in its (qi, x') band pattern, so it is
folded into the QK matmul as 17 extra contraction rows (PC = 96 + 17 = 113):
qblk rows 96.. hold the SVD left factor, kwin rows 96.. hold -BIG * right
factor. exp() then directly yields masked attention weights (0 out-of-window).

Pipeline per tile:
  PE : scoresT chunk [81, 96(h,q)] = kwin_chunk.T @ qblk  (mask folded in)
  ACT: amT = exp(0.25 * scoresT)  (PSUM -> SBUF)  == masked attn (unnormalized)
  PE : pa [96, 97] += amT_chunk.T @ vtT_chunk  (col 96 = denominator)
  DVE: g = pa[:, :96] * head-block mask; o16 = strided head-fold; rc = 1/denom
  ACT: out = o16 * rc
"""
import os
import numpy as np

B, d, D, H, W = 2, 96, 16, 32, 32
NH, HD = 6, 16
DL, DLH = 4, 6
NT = DL * H * 2
F = 162
BIG = 200.0

_cache = {}


def _mask_factors():
    """band[qi, x'] = 1 iff x'-qi in {0,1,2}; factor -BIG*(1-band) = A @ Bf."""
    band = np.zeros((16, 18), np.float64)
    for qi in range(16):
        band[qi, qi:qi + 3] = 1.0
    M = 1.0 - band
    U, S, Vt = np.linalg.svd(M)
    r = int(np.sum(S > 1e-9))
    A = U[:, :r] * np.sqrt(S[:r])            # [16, r]
    Bf = (np.sqrt(S[:r])[:, None] * Vt[:r])  # [r, 18]
    assert np.abs(A @ Bf - M).max() < 1e-6
    return A, -BIG * Bf, r


_A, _Bf, _R = _mask_factors()
PC = 96 + _R                                  # contraction partitions


def _build_nc():
    from concourse import bacc, mybir
    import concourse.tile as tile
    from contextlib import ExitStack

    f32 = mybir.dt.float32
    nc = bacc.Bacc(None, target_bir_lowering=False, debug=True)

    qblk_d = nc.declare_dram_parameter("qblk", [NT, PC, 96], f32, isOutput=False)
    kwin_d = nc.declare_dram_parameter("kwin", [NT, PC, F], f32, isOutput=False)
    vt_d = nc.declare_dram_parameter("vt", [NT, 81, 2, 97], f32, isOutput=False)
    m2_d = nc.declare_dram_parameter("m2", [96, 96], f32, isOutput=False)
    out_d = nc.declare_dram_parameter("out", [NT, 96, 16], f32, isOutput=True)

    with ExitStack() as ctx:
        tc = ctx.enter_context(tile.TileContext(nc))
        cpool = ctx.enter_context(tc.tile_pool(name="consts", bufs=1))
        qpool = ctx.enter_context(tc.tile_pool(name="q", bufs=4))
        kpool = ctx.enter_context(tc.tile_pool(name="kw", bufs=4))
        epool = ctx.enter_context(tc.tile_pool(name="es", bufs=4))
        vpool = ctx.enter_context(tc.tile_pool(name="vt", bufs=4))
        gpool = ctx.enter_context(tc.tile_pool(name="g", bufs=4))
        opool = ctx.enter_context(tc.tile_pool(name="o", bufs=4))
        pspool = ctx.enter_context(tc.tile_pool(name="ps", bufs=2, space="PSUM"))
        papool = ctx.enter_context(tc.tile_pool(name="pa", bufs=2, space="PSUM"))

        m2_sb = cpool.tile([96, 96], f32)
        nc.sync.dma_start(m2_sb[:], m2_d[:])

        for t in range(NT):
            qb = qpool.tile([PC, 96], f32, tag="qb")
            nc.sync.dma_start(qb[:], qblk_d[t])
            kw = kpool.tile([PC, F], f32, tag="kw")
            nc.sync.dma_start(kw[:], kwin_d[t])
            vt = vpool.tile([81, 2, 97], f32, tag="vt")
            nc.sync.dma_start(vt[:], vt_d[t])

            pa = papool.tile([96, 97], f32, tag="pa")
            for c in range(2):
                ps = pspool.tile([81, 96], f32, tag=f"ps{c}")
                nc.tensor.matmul(
                    ps[:], lhsT=kw[:, 81 * c:81 * c + 81], rhs=qb[:],
                    start=True, stop=True,
                )
                amt = epool.tile([81, 96], f32, tag=f"amt{c}")
                nc.scalar.activation(
                    amt[:], ps[:], mybir.ActivationFunctionType.Exp, scale=0.25
                )
                nc.tensor.matmul(
                    pa[:], lhsT=amt[:], rhs=vt[:, c, :],
                    start=(c == 0), stop=(c == 1),
                )

            g = gpool.tile([96, 96], f32, tag="g")
            nc.vector.scalar_tensor_tensor(
                g[:], pa[:, 0:96], 1.0, m2_sb[:],
                op0=mybir.AluOpType.mult, op1=mybir.AluOpType.mult,
            )
            o16 = opool.tile([96, 16], f32, tag="o16")
            nc.vector.tensor_reduce(
                o16[:], g.rearrange("p (h c) -> p c h", h=NH),
                axis=mybir.AxisListType.X, op=mybir.AluOpType.add,
            )
            rc = opool.tile([96, 1], f32, tag="rc")
            nc.vector.reciprocal(rc[:], pa[:, 96:97])
            of = opool.tile([96, 16], f32, tag="of")
            nc.scalar.activation(
                of[:], o16[:], mybir.ActivationFunctionType.Copy, scale=rc[:]
            )
            nc.sync.dma_start(out_d[t], of[:])
    nc.compile()
    return nc


def _consts():
    return np.kron(np.eye(NH, dtype=np.float32), np.ones((16, 16), np.float32))


def _host_prep(q, k, v, b, dc):
    kp = np.pad(k[b], ((0, 0), (1, 1), (1, 1), (1, 1)))
    vp = np.pad(v[b], ((0, 0), (1, 1), (1, 1), (1, 1)))
    k_slab = kp[:, 4 * dc:4 * dc + DLH]            # [96, 6, 34, 34]
    v_slab = vp[:, 4 * dc:4 * dc + DLH]

    qr = q[b].reshape(NH, HD, D, H, W)[:, :, 4 * dc:4 * dc + DL]
    qr = qr.reshape(NH, HD, DL, H, 2, 16)
    qblk = np.zeros((DL, H, 2, PC, 96), np.float32)
    for h in range(NH):
        qblk[:, :, :, 16 * h:16 * h + 16, 16 * h:16 * h + 16] = \
            qr[h].transpose(1, 2, 3, 0, 4)
    # mask left factor rows: qblk[96+r, (h,qi)] = A[qi, r]
    qa = np.tile(_A.T.astype(np.float32).reshape(_R, 1, 16), (1, NH, 1))
    qblk[:, :, :, 96:, :] = qa.reshape(_R, 96)
    qblk = qblk.reshape(NT, PC, 96)

    def windows(slab):                              # -> [DL, H, 2, 96, 3, 3, 18]
        swv = np.lib.stride_tricks.sliding_window_view(
            slab, (3, 3, 18), axis=(1, 2, 3))       # [96, 4, 32, 17, 3, 3, 18]
        return swv[:, :, :, ::16].transpose(1, 2, 3, 0, 4, 5, 6)

    kwin = np.empty((DL, H, 2, PC, 3, 3, 18), np.float32)
    kwin[:, :, :, :96] = windows(k_slab)
    # mask right factor rows: kwin[96+r, (dz',y',x')] = -BIG*Bf[r, x']
    kwin[:, :, :, 96:] = _Bf.astype(np.float32).reshape(_R, 1, 1, 18)
    # reorder keys to (chunk, dz', y', x'9) so flat 81-chunks match vt boxes
    kwin = kwin.reshape(DL, H, 2, PC, 3, 3, 2, 9).transpose(0, 1, 2, 3, 6, 4, 5, 7)
    kwin = kwin.reshape(NT, PC, F)

    wv = windows(v_slab)                            # [DL, H, 2, 96, 3, 3, 18]
    vt = np.ones((DL, H, 2, 2, 81, 97), np.float32)
    wvt = wv.transpose(0, 1, 2, 4, 5, 6, 3)         # [DL,H,2,3,3,18,96]
    vt[..., 0, :, :96] = wvt[..., 0:9, :].reshape(DL, H, 2, 81, 96)
    vt[..., 1, :, :96] = wvt[..., 9:18, :].reshape(DL, H, 2, 81, 96)
    vt = vt.transpose(0, 1, 2, 4, 3, 5)             # [DL,H,2,81,2,97]
    return qblk, np.ascontiguousarray(kwin), \
        np.ascontiguousarray(vt.reshape(NT, 81, 2, 97))


def kernel(q, k, v):
    q = np.asarray(q, np.float32)
    k = np.asarray(k, np.float32)
    v = np.asarray(v, np.float32)

    if "nc" not in _cache:
        _cache["nc"] = _build_nc()
    nc = _cache["nc"]

    from concourse.bass_utils import run_bass_kernel_spmd

    m2 = _consts()
    in_maps = []
    for core in range(8):
        b, dc = divmod(core, 4)
        qblk, kwin, vt = _host_prep(q, k, v, b, dc)
        in_maps.append({"qblk": qblk, "kwin": kwin, "vt": vt, "m2": m2})

    res = run_bass_kernel_spmd(nc, in_maps, list(range(8)),
                               trace=bool(int(os.environ.get("KTRACE", "0"))))
    _cache["last_results"] = res

    full = np.zeros((B, D, H, W, d), np.float32)
    for core in range(8):
        b, dc = divmod(core, 4)
        o = res.results[core]["out"]
        o = o.reshape(DL, H, 2, NH, 16, 16).transpose(0, 1, 2, 4, 3, 5)
        full[b, 4 * dc:4 * dc + DL] = o.reshape(DL, H, W, d)
    return full
